# revision 7
# baseline (speedup 1.0000x reference)
"""Cross-attention kernel for Trainium2, 8-core SPMD.

Math (per batch b):
  Q = wq . x + bq            [1,H,W]   (1x1 conv over C=8)
  K = wk . y + bk            [1,H,W]
  V = wv . y + bv            [C,H,W]
  att = softmax(Q K^T / sqrt(W))       [H,H]
  out = gamma * (att @ V) + x

Sharding: 8 cores = 4 batches x 2 row-halves (i in [0,512) / [512,1024)).

Per-core pipeline (all big matmuls bf16 with fp32 PSUM accumulation):
  1. y loaded once, cast fp32->bf16 during DMA (SWDGE), resident in SBUF.
  2. Kt[w,j] / Qt[w,i] built by PE "diagonal" matmuls: lhsT = y/x block,
     rhs = wk_c*I  => accumulates sum_c wk_c * block^T directly (fused
     projection + transpose). Bias added in the PSUM->SBUF ACT copy.
  3. scoresT[j,i] = Kt^T Qt accumulated over w tiles; exp via ACT with
     scale=1/32 (softmax normalization deferred).
  4. Row sums via ones-vector matmul; transposed to [i-part] via tiny
     outer-product matmuls; scale_i = gamma / sums_i.
  5. S[c'] = expT^T @ y[c']  (attention applied to raw y channels;
     V projection folded out by associativity: att@V = (att@y).wv + bv).
  6. 8x8 channel mix on DVE/GPSIMD scalar_tensor_tensor chains (bf16).
  7. result = S_mix * scale_i + gamma*bv_c + x  (ACT + DVE), DMA out.
"""

import sys

sys.path.insert(0, "/opt/trn_rl_repo")

import numpy as np
import ml_dtypes

B, C, H, W = 4, 8, 1024, 1024
HH = H // 2  # rows per core
N_CORES = 8

_cache = {}


def _build_program():
    import concourse.bass as bass
    import concourse.mybir as mybir
    from concourse import bacc
    import concourse.tile as tile
    from contextlib import ExitStack

    f32 = mybir.dt.float32
    bf16 = mybir.dt.bfloat16

    nc = bacc.Bacc("TRN2", target_bir_lowering=False, debug=False,
                   num_devices=N_CORES)

    X = nc.dram_tensor("x", [C, HH, W], f32, kind="ExternalInput")
    Y = nc.dram_tensor("y", [C, H, W], f32, kind="ExternalInput")
    WD = nc.dram_tensor("wd", [16, 128, 128], bf16, kind="ExternalInput")
    CB = nc.dram_tensor("cb", [128, 16], f32, kind="ExternalInput")
    WVS = nc.dram_tensor("wvs", [128, 64], f32, kind="ExternalInput")
    O = nc.dram_tensor("o", [C, HH, W], f32, kind="ExternalOutput")

    xap, yap, oap = X.ap(), Y.ap(), O.ap()

    JT = H // 128      # 8 j tiles
    IT = HH // 128     # 4 i tiles
    WB = W // 128      # 8 w blocks
    MIX_DVE = set(range(8))   # gpsimd lacks TensorScalarPtr; all on DVE

    with ExitStack() as top:
        tc = top.enter_context(tile.TileContext(nc))
        const = top.enter_context(tc.tile_pool(name="const", bufs=1))
        ybp = top.enter_context(tc.tile_pool(name="ybp", bufs=1))
        expp = top.enter_context(tc.tile_pool(name="expp", bufs=1))
        smp = top.enter_context(tc.tile_pool(name="smp", bufs=1))

        # constants
        cb = const.tile([128, 16], f32, tag="cb")
        nc.sync.dma_start(out=cb, in_=CB.ap())
        wvs = const.tile([128, 64], f32, tag="wvs")
        nc.sync.dma_start(out=wvs, in_=WVS.ap())
        wd = []
        for k in range(16):
            t = const.tile([128, 128], bf16, tag=f"wd{k}")
            nc.sync.dma_start(out=t, in_=WD.ap()[k])
            wd.append(t)
        ones_bf = const.tile([128, 1], bf16, tag="ones")
        nc.vector.memset(ones_bf, 1.0)
        # cb columns: 0=bk 1=bq 2=gamma 3=1.0 4..11=gamma*bv[c]
        BK, BQ, GAM, ONE = 0, 1, 2, 3

        # resident y (bf16, cast during DMA)
        yb = [[None] * JT for _ in range(C)]
        for c in range(C):
            for jt in range(JT):
                t = ybp.tile([128, W], bf16, tag=f"y{c}_{jt}")
                nc.gpsimd.dma_start(out=t, in_=yap[c, jt * 128:(jt + 1) * 128, :])
                yb[c][jt] = t

        expT = [expp.tile([128, HH], bf16, tag=f"e{jt}", name=f"e{jt}") for jt in range(JT)]
        sums_sb = smp.tile([1, HH], f32, tag="sums")
        scale_sb = smp.tile([128, IT], f32, tag="scale")

        # ---- phase 1+2: projections (fused transpose), scores, exp, sums
        with ExitStack() as ph1:
            kqp = ph1.enter_context(tc.tile_pool(name="kqp", bufs=1))
            xwp = ph1.enter_context(tc.tile_pool(name="xwp", bufs=2))
            pp1 = ph1.enter_context(tc.tile_pool(name="pp1", bufs=2, space="PSUM"))
            psc = ph1.enter_context(tc.tile_pool(name="psc", bufs=2, space="PSUM"))
            psum_small = ph1.enter_context(
                tc.tile_pool(name="pss", bufs=1, space="PSUM"))

            # Kt[wb] : [128 w, H j]
            kt = [kqp.tile([128, H], bf16, tag=f"kt{wb}", name=f"kt{wb}") for wb in range(WB)]
            for jt in range(JT):
                for wb in range(WB):
                    pk = pp1.tile([128, 128], f32, tag="pk")
                    for c in range(C):
                        nc.tensor.matmul(
                            pk, yb[c][jt][:, wb * 128:(wb + 1) * 128], wd[c],
                            start=(c == 0), stop=(c == C - 1))
                    nc.scalar.activation(
                        kt[wb][:, jt * 128:(jt + 1) * 128], pk,
                        mybir.ActivationFunctionType.Identity,
                        bias=cb[:, BK:BK + 1], scale=1.0)

            # Qt[wb] : [128 w, HH i]
            qt = [kqp.tile([128, HH], bf16, tag=f"qt{wb}", name=f"qt{wb}") for wb in range(WB)]
            for it in range(IT):
                xbf = []
                for c in range(C):
                    xf = xwp.tile([128, W], f32, tag="xf")
                    nc.sync.dma_start(
                        out=xf, in_=xap[c, it * 128:(it + 1) * 128, :])
                    xb = xwp.tile([128, W], bf16, tag=f"xb{c}")
                    nc.scalar.activation(
                        xb, xf, mybir.ActivationFunctionType.Copy, scale=1.0)
                    xbf.append(xb)
                for wb in range(WB):
                    pq = pp1.tile([128, 128], f32, tag="pq")
                    for c in range(C):
                        nc.tensor.matmul(
                            pq, xbf[c][:, wb * 128:(wb + 1) * 128], wd[8 + c],
                            start=(c == 0), stop=(c == C - 1))
                    nc.scalar.activation(
                        qt[wb][:, it * 128:(it + 1) * 128], pq,
                        mybir.ActivationFunctionType.Identity,
                        bias=cb[:, BQ:BQ + 1], scale=1.0)

            # scoresT + exp + sums
            psums = psum_small.tile([1, HH], f32, tag="psums")
            for jt in range(JT):
                ps = psc.tile([128, HH], f32, tag="ps")
                for wb in range(WB):
                    nc.tensor.matmul(
                        ps, kt[wb][:, jt * 128:(jt + 1) * 128], qt[wb],
                        start=(wb == 0), stop=(wb == WB - 1))
                nc.scalar.activation(
                    expT[jt], ps, mybir.ActivationFunctionType.Exp,
                    scale=float(1.0 / np.sqrt(W)))
                nc.tensor.matmul(psums, ones_bf, expT[jt],
                                 start=(jt == 0), stop=(jt == JT - 1))

            nc.vector.tensor_copy(sums_sb, psums)
            psig = psum_small.tile([128, IT], f32, tag="psig")
            for t in range(IT):
                nc.tensor.matmul(
                    psig[:, t:t + 1], sums_sb[0:1, t * 128:(t + 1) * 128],
                    cb[0:1, ONE:ONE + 1], start=True, stop=True)
            sig = smp.tile([128, IT], f32, tag="sig")
            nc.vector.reciprocal(sig, psig)
            nc.vector.tensor_scalar(scale_sb, sig, cb[:, GAM:GAM + 1], None,
                                    op0=mybir.AluOpType.mult)

        # ---- phase 3: S = expT^T @ y, mix, scale, residual, store
        with ExitStack() as ph3:
            ssp = ph3.enter_context(tc.tile_pool(name="ssp", bufs=1))
            mxp = ph3.enter_context(tc.tile_pool(name="mxp", bufs=1))
            xrp = ph3.enter_context(tc.tile_pool(name="xrp", bufs=1))
            rsp = ph3.enter_context(tc.tile_pool(name="rsp", bufs=3))
            pS = ph3.enter_context(tc.tile_pool(name="pS", bufs=1, space="PSUM"))

            for it in range(IT):
                for wh in range(2):
                    s_sb = []
                    for cp in range(C):
                        psS = pS.tile([128, 512], f32, tag=f"pS{cp}")
                        for jt in range(JT):
                            nc.tensor.matmul(
                                psS,
                                expT[jt][:, it * 128:(it + 1) * 128],
                                yb[cp][jt][:, wh * 512:(wh + 1) * 512],
                                start=(jt == 0), stop=(jt == JT - 1))
                        sb_t = ssp.tile([128, 512], bf16, tag=f"s{cp}")
                        nc.scalar.activation(
                            sb_t, psS, mybir.ActivationFunctionType.Copy,
                            scale=1.0)
                        s_sb.append(sb_t)

                    xr = []
                    for c in range(C):
                        t = xrp.tile([128, 512], f32, tag=f"xr{c}")
                        nc.sync.dma_start(
                            out=t,
                            in_=xap[c, it * 128:(it + 1) * 128,
                                    wh * 512:(wh + 1) * 512])
                        xr.append(t)

                    for c in range(C):
                        eng = nc.vector if c in MIX_DVE else nc.gpsimd
                        acc = mxp.tile([128, 512], bf16, tag=f"mx{c}")
                        eng.tensor_scalar(
                            acc, s_sb[0], wvs[:, 8 * c:8 * c + 1], None,
                            op0=mybir.AluOpType.mult)
                        for cp in range(1, C):
                            eng.scalar_tensor_tensor(
                                acc, s_sb[cp], wvs[:, 8 * c + cp:8 * c + cp + 1],
                                acc, op0=mybir.AluOpType.mult,
                                op1=mybir.AluOpType.add)
                        u = mxp.tile([128, 512], bf16, tag=f"u{c}")
                        nc.scalar.activation(
                            u, acc, mybir.ActivationFunctionType.Identity,
                            bias=cb[:, 4 + c:5 + c],
                            scale=scale_sb[:, it:it + 1])
                        res = rsp.tile([128, 512], f32, tag="res")
                        nc.vector.tensor_tensor(
                            res, u, xr[c], op=mybir.AluOpType.add)
                        nc.sync.dma_start(
                            out=oap[c, it * 128:(it + 1) * 128,
                                    wh * 512:(wh + 1) * 512],
                            in_=res)
    nc.compile()
    return nc


def _consts(wq, bq, wk, bk, wv, bv, gamma):
    bf = ml_dtypes.bfloat16
    ident = np.eye(128, dtype=np.float32)
    wd = np.zeros((16, 128, 128), dtype=bf)
    for c in range(8):
        wd[c] = (ident * float(wk[0, c])).astype(bf)
        wd[8 + c] = (ident * float(wq[0, c])).astype(bf)
    cb = np.zeros((128, 16), dtype=np.float32)
    cb[:, 0] = float(bk[0])
    cb[:, 1] = float(bq[0])
    cb[:, 2] = float(gamma[0])
    cb[:, 3] = 1.0
    for c in range(8):
        cb[:, 4 + c] = float(gamma[0]) * float(bv[c])
    wvs = np.zeros((128, 64), dtype=np.float32)
    for c in range(8):
        for cp in range(8):
            wvs[:, 8 * c + cp] = float(wv[c, cp])
    return wd, cb, wvs


def kernel(x, y, wq, bq, wk, bk, wv, bv, gamma):
    from concourse.bass_utils import run_bass_kernel_spmd

    x = np.asarray(x, dtype=np.float32)
    y = np.asarray(y, dtype=np.float32)
    wd, cb, wvs = _consts(np.asarray(wq), np.asarray(bq), np.asarray(wk),
                          np.asarray(bk), np.asarray(wv), np.asarray(bv),
                          np.asarray(gamma))

    if "nc" not in _cache:
        _cache["nc"] = _build_program()
    nc = _cache["nc"]

    in_maps = []
    for core in range(N_CORES):
        b, h = core // 2, core % 2
        in_maps.append({
            "x": np.ascontiguousarray(x[b, :, h * HH:(h + 1) * HH, :]),
            "y": np.ascontiguousarray(y[b]),
            "wd": wd, "cb": cb, "wvs": wvs,
        })
    _cache["in_maps"] = in_maps
    global _last_in_maps
    _last_in_maps = in_maps
    res = run_bass_kernel_spmd(nc, in_maps, list(range(N_CORES)))
    out = np.empty((B, C, H, W), dtype=np.float32)
    for core in range(N_CORES):
        b, h = core // 2, core % 2
        out[b, :, h * HH:(h + 1) * HH, :] = res.results[core]["o"]
    return out
